# revision 2
# baseline (speedup 1.0000x reference)
"""Ternary-weight linear layer on 8 Trainium2 NeuronCores.

Problem: y = x @ ternarize(W).T + b
  x [8192, 4096] fp32, W [4096, 4096] fp32, b [4096] fp32.
  ternarize(w) = round(clamp(w, -1, 1))  (round-half-even, forward value).

Strategy (data-parallel over tokens, replicated weights):
  - Each of the 8 cores gets 1024 tokens. Host passes x and W already
    transposed (pure layout prep) so the contraction dim i lands on SBUF
    partitions with no on-device transposes:
        xT  [4096 i, 1024 t]  (per-core slice, fed as float32r)
        wT  [4096 i, 4096 o]  (replicated)
  - On device, W tiles are ternarized exactly with two chained DVE
    tensor_scalar ops: clamp via min/max, then round-half-even via the
    +C/-C trick (C = 1.5 * 2^23), output dtype float32r.
  - Matmuls run in fp32r (1 cycle/row on the PE, 4x faster than fp32):
    out[o,t] tile = sum_i wT_tile[i,o].T @ xT[i,t]. x is consumed as raw
    fp32 bits declared float32r; the PE applies its internal fp32r
    rounding (~13 mantissa bits), giving ~1e-4 relative error.
  - Bias is added during PSUM->SBUF eviction on the scalar engine
    (activation Copy with per-partition bias).
  - Per-core output is yT [4096 o, 1024 t]; the host transposes and
    concatenates (layout-only unshard).
"""

import numpy as np

N_CORES = 8
TOKENS = 8192
IN_F = 4096
OUT_F = 4096
T_CORE = TOKENS // N_CORES       # 1024 tokens per core
P = 128                          # partitions
KB = IN_F // P                   # 32 contraction blocks
TN = 512                         # moving free dim per matmul (1 PSUM bank)
TH = T_CORE // TN                # 2 t-halves
O_CHUNK = 256                    # o columns ternarized/matmul'd per pass
OB_PER_CHUNK = O_CHUNK // P      # 2
N_CHUNKS = OUT_F // O_CHUNK      # 16

C_ROUND = 12582912.0             # 1.5 * 2^23; (x+C)-C == round-half-even(x), |x|<=1

_built = None


def _build():
    import concourse.bacc as bacc
    import concourse.mybir as mybir
    import concourse.tile as tile

    dt = mybir.dt

    nc = bacc.Bacc("TRN2", target_bir_lowering=False, debug=False)
    xT_d = nc.dram_tensor("xT", [IN_F, T_CORE], dt.float32r, kind="ExternalInput").ap()
    wT_d = nc.dram_tensor("wT", [IN_F, OUT_F], dt.float32, kind="ExternalInput").ap()
    biasT_d = nc.dram_tensor("biasT", [P, OUT_F // P], dt.float32, kind="ExternalInput").ap()
    yT_d = nc.dram_tensor("yT", [OUT_F, T_CORE], dt.float32, kind="ExternalOutput").ap()

    with tile.TileContext(nc) as tc:
        with tc.tile_pool(name="xp", bufs=1) as xp, \
             tc.tile_pool(name="wp", bufs=4) as wp, \
             tc.tile_pool(name="wc", bufs=3) as wc, \
             tc.tile_pool(name="wt", bufs=4) as wtp, \
             tc.tile_pool(name="op", bufs=3) as op, \
             tc.tile_pool(name="cn", bufs=1) as cn, \
             tc.tile_pool(name="ps", bufs=2, space="PSUM") as ps:

            biasT = cn.tile([P, OUT_F // P], dt.float32, name="biasT_s")
            nc.sync.dma_start(out=biasT[:], in_=biasT_d[:])

            # x resident in SBUF: 32 tiles [128, 1024] fp32r (16 MB)
            xt = []
            for kb in range(KB):
                t = xp.tile([P, T_CORE], dt.float32r, tag=f"x{kb}", name=f"x{kb}")
                nc.sync.dma_start(out=t[:], in_=xT_d[kb * P:(kb + 1) * P, :])
                xt.append(t)

            for ch in range(N_CHUNKS):
                o0 = ch * O_CHUNK
                psums = [
                    ps.tile([P, TN], dt.float32, tag=f"ps{ob}_{th}",
                            name=f"ps_{ch}_{ob}_{th}")
                    for ob in range(OB_PER_CHUNK) for th in range(TH)
                ]
                for kb in range(KB):
                    wtile = wp.tile([P, O_CHUNK], dt.float32, tag="w",
                                    name=f"w_{ch}_{kb}")
                    nc.sync.dma_start(
                        out=wtile[:],
                        in_=wT_d[kb * P:(kb + 1) * P, o0:o0 + O_CHUNK])
                    wcl = wc.tile([P, O_CHUNK], dt.float32, tag="wcl",
                                  name=f"wcl_{ch}_{kb}")
                    nc.vector.tensor_scalar(wcl[:], wtile[:], 1.0, -1.0,
                                            mybir.AluOpType.min,
                                            mybir.AluOpType.max)
                    wter = wtp.tile([P, O_CHUNK], dt.float32r, tag="wter",
                                    name=f"wter_{ch}_{kb}")
                    nc.vector.tensor_scalar(wter[:], wcl[:], C_ROUND, C_ROUND,
                                            mybir.AluOpType.add,
                                            mybir.AluOpType.subtract)
                    first, last = kb == 0, kb == KB - 1
                    for ob in range(OB_PER_CHUNK):
                        lhsT = wter[:, ob * P:(ob + 1) * P]
                        for th in range(TH):
                            nc.tensor.matmul(
                                psums[ob * TH + th][:],
                                lhsT,
                                xt[kb][:, th * TN:(th + 1) * TN],
                                start=first, stop=last)

                # evict PSUM -> SBUF with fused bias add, then DMA out
                for ob in range(OB_PER_CHUNK):
                    o_abs = o0 + ob * P
                    stage = op.tile([P, T_CORE], dt.float32, tag="out",
                                    name=f"out_{ch}_{ob}")
                    for th in range(TH):
                        nc.scalar.activation(
                            stage[:, th * TN:(th + 1) * TN],
                            psums[ob * TH + th][:],
                            mybir.ActivationFunctionType.Identity,
                            bias=biasT[:, o_abs // P:o_abs // P + 1],
                            scale=1.0)
                    nc.sync.dma_start(
                        out=yT_d[o_abs:o_abs + P, :], in_=stage[:])

    nc.compile()
    return nc


def kernel(input, weight, bias):
    global _built
    if _built is None:
        _built = _build()
    nc = _built
    from concourse.bass_utils import run_bass_kernel_spmd

    input = np.ascontiguousarray(input, dtype=np.float32)
    weight = np.ascontiguousarray(weight, dtype=np.float32)
    bias = np.ascontiguousarray(bias, dtype=np.float32)

    wT = np.ascontiguousarray(weight.T)                      # [i, o]
    biasT = np.ascontiguousarray(bias.reshape(OUT_F // P, P).T)  # [128, 32]

    in_maps = []
    for c in range(N_CORES):
        x_c = input[c * T_CORE:(c + 1) * T_CORE]             # [1024, 4096]
        xT_c = np.ascontiguousarray(x_c.T)                   # [4096, 1024]
        in_maps.append({"xT": xT_c, "wT": wT, "biasT": biasT})

    res = run_bass_kernel_spmd(nc, in_maps, list(range(N_CORES)))

    y = np.empty((TOKENS, OUT_F), dtype=np.float32)
    for c in range(N_CORES):
        y[c * T_CORE:(c + 1) * T_CORE] = res.results[c]["yT"].T
    return y


# revision 4
# speedup vs baseline: 95.0035x; 95.0035x over previous
"""Ternary-weight linear layer on 8 Trainium2 NeuronCores.

Problem: y = x @ ternarize(W).T + b
  x [8192, 4096] fp32, W [4096, 4096] fp32, b [4096] fp32.
  ternarize(w) = round(clamp(w, -1, 1))  (round-half-even, forward value).

Strategy (data-parallel over tokens, replicated weights):
  - Each of the 8 cores gets 1024 tokens. Host passes x and W already
    transposed (pure layout prep) so the contraction dim i lands on SBUF
    partitions with no on-device transposes:
        xT  [4096 i, 1024 t]  (per-core slice, fed as float32r)
        wT  [4096 i, 4096 o]  (replicated)
  - On device, W tiles are ternarized exactly with two chained DVE
    tensor_scalar ops: clamp via min/max, then round-half-even via the
    +C/-C trick (C = 1.5 * 2^23), output dtype float32r.
  - Matmuls run in fp32r (1 cycle/row on the PE, 4x faster than fp32):
    out[o,t] tile = sum_i wT_tile[i,o].T @ xT[i,t]. x is consumed as raw
    fp32 bits declared float32r; the PE applies its internal fp32r
    rounding (~13 mantissa bits), giving ~1e-4 relative error.
  - Bias is added during PSUM->SBUF eviction on the scalar engine
    (activation Copy with per-partition bias).
  - Per-core output is yT [4096 o, 1024 t]; the host transposes and
    concatenates (layout-only unshard).
"""

import numpy as np

N_CORES = 8
TOKENS = 8192
IN_F = 4096
OUT_F = 4096
T_CORE = TOKENS // N_CORES       # 1024 tokens per core
P = 128                          # partitions
KB = IN_F // P                   # 32 contraction blocks
TN = 512                         # moving free dim per matmul (1 PSUM bank)
TH = T_CORE // TN                # 2 t-halves
O_CHUNK = 256                    # o columns ternarized/matmul'd per pass
OB_PER_CHUNK = O_CHUNK // P      # 2
N_CHUNKS = OUT_F // O_CHUNK      # 16

C_ROUND = 12582912.0             # 1.5 * 2^23; (x+C)-C == round-half-even(x), |x|<=1

_built = None


def _build(reps=1):
    import contextlib
    import concourse.bacc as bacc
    import concourse.mybir as mybir
    import concourse.tile as tile

    dt = mybir.dt

    nc = bacc.Bacc("TRN2", target_bir_lowering=False, debug=False)
    xT_d = nc.dram_tensor("xT", [IN_F, T_CORE], dt.float32r, kind="ExternalInput").ap()
    wT_d = nc.dram_tensor("wT", [IN_F, OUT_F], dt.float32, kind="ExternalInput").ap()
    biasT_d = nc.dram_tensor("biasT", [P, OUT_F // P], dt.float32, kind="ExternalInput").ap()
    yT_d = nc.dram_tensor("yT", [OUT_F, T_CORE], dt.float32, kind="ExternalOutput").ap()

    with tile.TileContext(nc) as tc:
        with tc.tile_pool(name="xp", bufs=1) as xp, \
             tc.tile_pool(name="wp", bufs=4) as wp, \
             tc.tile_pool(name="wc", bufs=3) as wc, \
             tc.tile_pool(name="wt", bufs=4) as wtp, \
             tc.tile_pool(name="op", bufs=3) as op, \
             tc.tile_pool(name="cn", bufs=1) as cn, \
             tc.tile_pool(name="ps", bufs=2, space="PSUM") as ps:

            biasT = cn.tile([P, OUT_F // P], dt.float32, name="biasT_s")
            nc.sync.dma_start(out=biasT[:], in_=biasT_d[:])

            # x resident in SBUF: 32 tiles [128, 1024] fp32r (16 MB)
            xt = []
            for kb in range(KB):
                t = xp.tile([P, T_CORE], dt.float32r, tag=f"x{kb}", name=f"x{kb}")
                nc.sync.dma_start(out=t[:], in_=xT_d[kb * P:(kb + 1) * P, :])
                xt.append(t)

            rep_ctx = tc.For_i(0, reps, 1) if reps > 1 else contextlib.nullcontext()
            with rep_ctx:
              for ch in range(N_CHUNKS):
                o0 = ch * O_CHUNK
                psums = [
                    ps.tile([P, TN], dt.float32, tag=f"ps{ob}_{th}",
                            name=f"ps_{ch}_{ob}_{th}")
                    for ob in range(OB_PER_CHUNK) for th in range(TH)
                ]
                for kb in range(KB):
                    wtile = wp.tile([P, O_CHUNK], dt.float32, tag="w",
                                    name=f"w_{ch}_{kb}")
                    nc.sync.dma_start(
                        out=wtile[:],
                        in_=wT_d[kb * P:(kb + 1) * P, o0:o0 + O_CHUNK])
                    wcl = wc.tile([P, O_CHUNK], dt.float32, tag="wcl",
                                  name=f"wcl_{ch}_{kb}")
                    nc.vector.tensor_scalar(wcl[:], wtile[:], 1.0, -1.0,
                                            mybir.AluOpType.min,
                                            mybir.AluOpType.max)
                    wter = wtp.tile([P, O_CHUNK], dt.float32r, tag="wter",
                                    name=f"wter_{ch}_{kb}")
                    nc.vector.tensor_scalar(wter[:], wcl[:], C_ROUND, C_ROUND,
                                            mybir.AluOpType.add,
                                            mybir.AluOpType.subtract)
                    first, last = kb == 0, kb == KB - 1
                    for ob in range(OB_PER_CHUNK):
                        lhsT = wter[:, ob * P:(ob + 1) * P]
                        for th in range(TH):
                            nc.tensor.matmul(
                                psums[ob * TH + th][:],
                                lhsT,
                                xt[kb][:, th * TN:(th + 1) * TN],
                                start=first, stop=last)

                # evict PSUM -> SBUF with fused bias add, then DMA out
                for ob in range(OB_PER_CHUNK):
                    o_abs = o0 + ob * P
                    stage = op.tile([P, T_CORE], dt.float32, tag="out",
                                    name=f"out_{ch}_{ob}")
                    for th in range(TH):
                        nc.scalar.activation(
                            stage[:, th * TN:(th + 1) * TN],
                            psums[ob * TH + th][:],
                            mybir.ActivationFunctionType.Identity,
                            bias=biasT[:, o_abs // P:o_abs // P + 1],
                            scale=1.0)
                    nc.sync.dma_start(
                        out=yT_d[o_abs:o_abs + P, :], in_=stage[:])

    nc.compile()
    return nc


def kernel(input, weight, bias):
    global _built
    if _built is None:
        _built = _build()
    nc = _built
    from concourse.bass_utils import run_bass_kernel_spmd

    input = np.ascontiguousarray(input, dtype=np.float32)
    weight = np.ascontiguousarray(weight, dtype=np.float32)
    bias = np.ascontiguousarray(bias, dtype=np.float32)

    wT = np.ascontiguousarray(weight.T)                      # [i, o]
    biasT = np.ascontiguousarray(bias.reshape(OUT_F // P, P).T)  # [128, 32]

    in_maps = []
    for c in range(N_CORES):
        x_c = input[c * T_CORE:(c + 1) * T_CORE]             # [1024, 4096]
        xT_c = np.ascontiguousarray(x_c.T)                   # [4096, 1024]
        in_maps.append({"xT": xT_c, "wT": wT, "biasT": biasT})

    res = run_bass_kernel_spmd(nc, in_maps, list(range(N_CORES)))

    y = np.empty((TOKENS, OUT_F), dtype=np.float32)
    for c in range(N_CORES):
        y[c * T_CORE:(c + 1) * T_CORE] = res.results[c]["yT"].T
    return y


# revision 5
# speedup vs baseline: 179.6653x; 1.8911x over previous
"""Ternary-weight linear layer on 8 Trainium2 NeuronCores.

Problem: y = x @ ternarize(W).T + b
  x [8192, 4096] fp32, W [4096, 4096] fp32, b [4096] fp32.
  ternarize(w) = round(clamp(w, -1, 1))  (round-half-even, forward value).

Strategy (data-parallel over tokens, replicated weights):
  - Each of the 8 cores gets 1024 tokens. Host passes x and W already
    transposed (pure layout prep) so the contraction dim i lands on SBUF
    partitions with no on-device transposes:
        xT  [4096 i, 1024 t]  (per-core slice)
        wT  [4096 i, 4096 o]  (replicated)
  - On device, W tiles are ternarized exactly with two chained DVE
    tensor_scalar ops: clamp via min/max, then round-half-even via the
    +C/-C trick (C = 1.5 * 2^23). Ternary values are exact in bf16.
  - mode "bf16x2": x is split on-device into x_hi = bf16(x) and
    x_lo = bf16(x - x_hi); two bf16 matmuls accumulate into the same
    PSUM bank. bf16 streams 1 cycle/column on the PE (measured 198
    ns per 512-col matmul) and exact ternary weights make the result
    accurate to ~2e-6 relative.
  - mode "f32r": single-pass float32r matmuls (measured 434 ns/MM =
    2 cycles/column, ~1e-4 relative error). Same speed as bf16x2 but
    less accurate; kept for comparison.
  - Bias is added during PSUM->SBUF eviction on the scalar engine
    (activation Identity with per-partition bias).
  - Per-core output is yT [4096 o, 1024 t]; the host transposes and
    concatenates (layout-only unshard).
"""

import numpy as np

N_CORES = 8
TOKENS = 8192
IN_F = 4096
OUT_F = 4096
T_CORE = TOKENS // N_CORES       # 1024 tokens per core
P = 128                          # partitions
KB = IN_F // P                   # 32 contraction blocks
TN = 512                         # moving free dim per matmul (1 PSUM bank)
TH = T_CORE // TN                # 2 t-halves
O_CHUNK = 256                    # o columns ternarized/matmul'd per pass
OB_PER_CHUNK = O_CHUNK // P      # 2
N_CHUNKS = OUT_F // O_CHUNK      # 16

C_ROUND = 12582912.0             # 1.5 * 2^23; (x+C)-C == round-half-even(x), |x|<=1

MODE = "bf16x2"                  # "bf16x2" | "f32r"

_built = None


def _build(reps=1, mode=MODE):
    import contextlib
    import concourse.bacc as bacc
    import concourse.mybir as mybir
    import concourse.tile as tile

    dt = mybir.dt
    x_in_dt = dt.float32 if mode == "bf16x2" else dt.float32r
    w_dt = dt.bfloat16 if mode == "bf16x2" else dt.float32r

    nc = bacc.Bacc("TRN2", target_bir_lowering=False, debug=False)
    xT_d = nc.dram_tensor("xT", [IN_F, T_CORE], x_in_dt, kind="ExternalInput").ap()
    wT_d = nc.dram_tensor("wT", [IN_F, OUT_F], dt.float32, kind="ExternalInput").ap()
    biasT_d = nc.dram_tensor("biasT", [P, OUT_F // P], dt.float32, kind="ExternalInput").ap()
    yT_d = nc.dram_tensor("yT", [OUT_F, T_CORE], dt.float32, kind="ExternalOutput").ap()

    with tile.TileContext(nc) as tc:
        with tc.tile_pool(name="xp", bufs=1) as xp, \
             tc.tile_pool(name="xi", bufs=3) as xi, \
             tc.tile_pool(name="wp", bufs=4) as wp, \
             tc.tile_pool(name="wc", bufs=3) as wc, \
             tc.tile_pool(name="wt", bufs=4) as wtp, \
             tc.tile_pool(name="op", bufs=3) as op, \
             tc.tile_pool(name="cn", bufs=1) as cn, \
             tc.tile_pool(name="ps", bufs=2, space="PSUM") as ps:

            biasT = cn.tile([P, OUT_F // P], dt.float32, name="biasT_s")
            nc.sync.dma_start(out=biasT[:], in_=biasT_d[:])

            # x resident in SBUF
            xsrc = []          # list of (hi, lo) or single fp32r tiles
            for kb in range(KB):
                sl = xT_d[kb * P:(kb + 1) * P, :]
                if mode == "bf16x2":
                    xtmp = xi.tile([P, T_CORE], dt.float32, tag="xtmp",
                                   name=f"xtmp{kb}")
                    nc.sync.dma_start(out=xtmp[:], in_=sl)
                    xhi = xp.tile([P, T_CORE], dt.bfloat16, tag=f"xh{kb}",
                                  name=f"xh{kb}")
                    nc.vector.tensor_copy(xhi[:], xtmp[:])
                    xlo = xp.tile([P, T_CORE], dt.bfloat16, tag=f"xl{kb}",
                                  name=f"xl{kb}")
                    nc.vector.tensor_sub(xlo[:], xtmp[:], xhi[:])
                    xsrc.append((xhi, xlo))
                else:
                    t = xp.tile([P, T_CORE], dt.float32r, tag=f"x{kb}",
                                name=f"x{kb}")
                    nc.sync.dma_start(out=t[:], in_=sl)
                    xsrc.append((t,))

            rep_ctx = tc.For_i(0, reps, 1) if reps > 1 else contextlib.nullcontext()
            with rep_ctx:
              for ch in range(N_CHUNKS):
                o0 = ch * O_CHUNK
                psums = [
                    ps.tile([P, TN], dt.float32, tag=f"ps{ob}_{th}",
                            name=f"ps_{ch}_{ob}_{th}")
                    for ob in range(OB_PER_CHUNK) for th in range(TH)
                ]
                for kb in range(KB):
                    wtile = wp.tile([P, O_CHUNK], dt.float32, tag="w",
                                    name=f"w_{ch}_{kb}")
                    nc.sync.dma_start(
                        out=wtile[:],
                        in_=wT_d[kb * P:(kb + 1) * P, o0:o0 + O_CHUNK])
                    wcl = wc.tile([P, O_CHUNK], dt.float32, tag="wcl",
                                  name=f"wcl_{ch}_{kb}")
                    nc.vector.tensor_scalar(wcl[:], wtile[:], 1.0, -1.0,
                                            mybir.AluOpType.min,
                                            mybir.AluOpType.max)
                    wter = wtp.tile([P, O_CHUNK], w_dt, tag="wter",
                                    name=f"wter_{ch}_{kb}")
                    nc.vector.tensor_scalar(wter[:], wcl[:], C_ROUND, C_ROUND,
                                            mybir.AluOpType.add,
                                            mybir.AluOpType.subtract)
                    first, last = kb == 0, kb == KB - 1
                    for ob in range(OB_PER_CHUNK):
                        lhsT = wter[:, ob * P:(ob + 1) * P]
                        for th in range(TH):
                            for xi_, xpart in enumerate(xsrc[kb]):
                                nc.tensor.matmul(
                                    psums[ob * TH + th][:],
                                    lhsT,
                                    xpart[:, th * TN:(th + 1) * TN],
                                    start=(first and xi_ == 0),
                                    stop=(last and xi_ == len(xsrc[kb]) - 1))

                # evict PSUM -> SBUF with fused bias add, then DMA out
                for ob in range(OB_PER_CHUNK):
                    o_abs = o0 + ob * P
                    stage = op.tile([P, T_CORE], dt.float32, tag="out",
                                    name=f"out_{ch}_{ob}")
                    for th in range(TH):
                        nc.scalar.activation(
                            stage[:, th * TN:(th + 1) * TN],
                            psums[ob * TH + th][:],
                            mybir.ActivationFunctionType.Identity,
                            bias=biasT[:, o_abs // P:o_abs // P + 1],
                            scale=1.0)
                    nc.sync.dma_start(
                        out=yT_d[o_abs:o_abs + P, :], in_=stage[:])

    nc.compile()
    return nc


def kernel(input, weight, bias):
    global _built
    if _built is None:
        _built = _build()
    nc = _built
    from concourse.bass_utils import run_bass_kernel_spmd

    input = np.ascontiguousarray(input, dtype=np.float32)
    weight = np.ascontiguousarray(weight, dtype=np.float32)
    bias = np.ascontiguousarray(bias, dtype=np.float32)

    wT = np.ascontiguousarray(weight.T)                      # [i, o]
    biasT = np.ascontiguousarray(bias.reshape(OUT_F // P, P).T)  # [128, 32]

    in_maps = []
    for c in range(N_CORES):
        x_c = input[c * T_CORE:(c + 1) * T_CORE]             # [1024, 4096]
        xT_c = np.ascontiguousarray(x_c.T)                   # [4096, 1024]
        in_maps.append({"xT": xT_c, "wT": wT, "biasT": biasT})

    res = run_bass_kernel_spmd(nc, in_maps, list(range(N_CORES)))

    y = np.empty((TOKENS, OUT_F), dtype=np.float32)
    for c in range(N_CORES):
        y[c * T_CORE:(c + 1) * T_CORE] = res.results[c]["yT"].T
    return y
